# revision 11
# baseline (speedup 1.0000x reference)
"""Multi-head attention (B=4, N=2048, C=1024, H=16) on 8 TRN2 NeuronCores.

Sharding: tensor-parallel over heads. Each core owns H/8 = 2 heads:
  - qkv projection: w_qkv column-sharded by head group, x replicated
    (passed pre-transposed as xT so the contraction dim lands on
    partitions). Q^T/K^T/V^T strips come out of 512-wide matmuls; V is
    re-transposed to natural layout on the PE (head-paired transposes run
    concurrently on disjoint row-groups), with a ones column appended.
  - attention: fully local per (batch, head). Scores are computed
    transposed (S^T[k, q]) so softmax sums reduce over the partition axis
    via the ones-column of V (PV output row 64 = softmax sums). Both
    heads' scores for one k-tile land in a single [128, 2048] bf16 PSUM
    tile via two 1024-wide matmuls emitted adjacently (disjoint PE row
    halves -> concurrent). Softmax exp is split across engines: ScalarE
    exps head 0's bank while VectorE computes head 1's exp with a
    Schraudolph bit-trick (round(a*s+b) as int16, bitcast to bf16) on the
    other PSUM bank. Softmax reciprocal is exp(-ln(sums)) on ScalarE
    (both functions pinned to one activation table set); the row is
    broadcast across partitions by a tiny ones-outer-product matmul and
    applied on VectorE.
  - projection: a per-batch AllToAll redistributes the attention output
    from column shards (head groups) to row shards (sequence slices);
    each core then computes its row slice of out @ w_proj + b_proj with
    the full w_proj replicated. The last batch instead runs one half-valid
    AllToAll per 1024-column chunk (invalid shards zero-filled, outputs
    summed via an accumulating DMA) so the kernel tail only waits on the
    final chunk's collective.

Everything is emitted under one TileContext; batches are software-
pipelined (qkv/attn of batch b overlap the collective of batch b-1 and
the projection of batch b-2).
"""


import numpy as np

import concourse.bass as bass
import concourse.mybir as mybir
import concourse.tile as tile
from concourse import bacc
from concourse.bass_utils import run_bass_kernel_spmd
from concourse.masks import make_identity

F32 = mybir.dt.float32
F32R = mybir.dt.float32r
BF16 = mybir.dt.bfloat16

DTYPE_MODE = "bf16"   # "bf16" | "f32r"

# Problem shape (hardcoded for the harness; build_nc is parameterized for
# small-shape simulation in test.py).
B, N, C, H = 4, 2048, 1024, 16
NCORES = 8


def _raw_matmul(nc, out, lhsT, rhs, start, stop):
    """nc.tensor.matmul minus the fp32-only output assert, so S^T scores
    can drain from PSUM directly as bf16 (1024-wide, one bank)."""
    eng = nc.tensor
    ifmap_ap = eng.lower_ap(rhs.opt({0}), opt=False)
    weights_ap = eng.lower_ap(lhsT.opt({0}), opt=False,
                              for_matmul_weights=True)
    out_ap = eng.lower_ap(out)

    def round_up_size(size):
        for v in (32, 64, 128):
            if v >= size:
                return v
        raise AssertionError(size)

    assert lhsT.base_partition() == rhs.base_partition()
    tile_position = (lhsT.base_partition(), out.base_partition())
    tile_size = (round_up_size(rhs.partition_size()),
                 round_up_size(out.partition_size()))
    return eng.add_instruction(mybir.InstMatmult(
        name=nc.get_next_instruction_name(),
        replication_resolution=0,
        replication_shift_amnt=0,
        replication_num_rows=0,
        start_tensor_calc=start,
        stop_tensor_calc=stop,
        ins=[ifmap_ap, weights_ap],
        outs=[out_ap],
        perf_mode=None,
        is_transpose=False,
        ifmap_quant_offset=None,
        weights_quant_offset=None,
        bass_skip_group_check=False,
        tile_position=tile_position,
        tile_size=tile_size,
    ))


def build_nc(b_sz=B, n_sz=N, c_sz=C, h_sz=H, ncores=NCORES, dtype_mode=None):
    HD = c_sz // h_sz                # head dim (64)
    HPC = h_sz // ncores             # heads per core (2)
    WC = HPC * HD                    # this core's out-column width (128)
    RB = n_sz // ncores              # rows per (core, batch) after A2A (256)
    SCALE = float(HD) ** -0.5

    RCW = 512                        # qkv row-chunk width
    NRC = n_sz // RCW
    QCW = 1024                       # attention q-chunk width (ACT exp FD)
    NQC = n_sz // QCW
    NKT = n_sz // 128                # k-tiles per sequence
    CCH = c_sz // 128                # contraction chunks over C
    MT = RB // 128                   # row tiles per (core, batch) in proj
    NCOL = c_sz // 512               # 512-wide output column chunks in proj

    assert n_sz % QCW == 0 and n_sz % RCW == 0 and RB % 128 == 0
    assert c_sz % 512 == 0 and WC == 128 and HD == 64

    dtype_mode = dtype_mode or DTYPE_MODE
    DTM = {"bf16": BF16, "f32r": F32R}[dtype_mode]
    DTT = BF16 if dtype_mode == "bf16" else F32   # transpose-path dtype

    # Pin every activation to the one table set containing both Exp and
    # Ln, so the Ln/Exp softmax-reciprocal never thrashes ACT table loads.
    from concourse import hw_specs
    tables = hw_specs.get_activation_tables("gen3")
    for name, fns in tables.items():
        if name != "natural_log_exp_and_others":
            fns.discard(mybir.ActivationFunctionType.Exp)
            fns.discard(mybir.ActivationFunctionType.Ln)

    nc = bacc.Bacc(
        "TRN2", target_bir_lowering=False, debug=False, num_devices=ncores
    )

    xT = nc.dram_tensor("xT", [c_sz, b_sz * n_sz], DTM, kind="ExternalInput").ap()
    wqkv = nc.dram_tensor("wqkv", [c_sz, 3 * WC], DTM, kind="ExternalInput").ap()
    wproj = nc.dram_tensor("wproj", [c_sz, c_sz], DTM, kind="ExternalInput").ap()
    bproj = nc.dram_tensor("bproj", [c_sz], F32, kind="ExternalInput").ap()
    out = nc.dram_tensor("out", [b_sz, RB, c_sz], F32, kind="ExternalOutput").ap()

    with tile.TileContext(nc) as tc:
        from contextlib import ExitStack

        with ExitStack() as ctx:
            singles = ctx.enter_context(tc.tile_pool(name="singles", bufs=1))
            xpool = ctx.enter_context(tc.tile_pool(name="xpool", bufs=2))
            qt_pool = ctx.enter_context(tc.tile_pool(name="qt", bufs=2))
            kt_pool = ctx.enter_context(tc.tile_pool(name="kt", bufs=2))
            vt_pool = ctx.enter_context(tc.tile_pool(name="vt", bufs=1))
            vaug_pool = ctx.enter_context(tc.tile_pool(name="vaug", bufs=2))
            es_pool = ctx.enter_context(tc.tile_pool(name="es", bufs=4))
            ot_pool = ctx.enter_context(tc.tile_pool(name="ot", bufs=3))
            eps_pool = ctx.enter_context(tc.tile_pool(name="eps", bufs=4))
            bc_pool = ctx.enter_context(tc.tile_pool(name="bc", bufs=2))
            recv_pool = ctx.enter_context(tc.tile_pool(name="recv", bufs=3))
            y_pool = ctx.enter_context(tc.tile_pool(name="y", bufs=3))
            st_pool = ctx.enter_context(
                tc.tile_pool(name="stp", bufs=2, space="PSUM")
            )
            po_pool = ctx.enter_context(
                tc.tile_pool(name="pop", bufs=2, space="PSUM")
            )
            dram_in = ctx.enter_context(
                tc.tile_pool(name="a2a_in", bufs=3, space="DRAM")
            )
            eps_dram = ctx.enter_context(
                tc.tile_pool(name="eps_dram", bufs=4, space="DRAM")
            )
            dram_out = ctx.enter_context(
                tc.tile_pool(name="a2a_out", bufs=6, space="DRAM")
            )

            # ---- constants / weights ----
            w_sb = singles.tile([128, CCH, 3 * WC], DTM)
            nc.sync.dma_start(
                out=w_sb, in_=wqkv.rearrange("(cc p) m -> p cc m", p=128)
            )
            identity = singles.tile([128, 128], DTT)
            make_identity(nc, identity)
            zero_sb = singles.tile([128, n_sz // ncores], DTM)
            nc.vector.memset(zero_sb, 0.0)

            a2a_outs = [None] * b_sz
            proj_consts = {}

            def ensure_proj_consts():
                # the projection weights are first needed two batches in -
                # defer their (2 MiB) load so batch 0's x chunks lead the
                # sync DMA queue at startup
                if proj_consts:
                    return
                wp_sb = singles.tile([128, CCH, c_sz], DTM)
                nc.sync.dma_start(
                    out=wp_sb, in_=wproj.rearrange("(cc p) m -> p cc m", p=128)
                )
                b_sb = singles.tile([128, c_sz], F32)
                nc.gpsimd.dma_start(
                    out=b_sb,
                    in_=bass.AP(
                        tensor=bproj.tensor,
                        offset=bproj.offset,
                        ap=[[0, 128]] + list(bproj.ap),
                    ),
                )
                proj_consts["wp"] = wp_sb
                proj_consts["b"] = b_sb

            def proj_phase(bb):
                ensure_proj_consts()
                wp_sb = proj_consts["wp"]
                b_sb = proj_consts["b"]
                src_b = a2a_outs[bb]
                if not isinstance(src_b, list):
                    src_b = [(None, None, src_b)]
                recvs = []
                for idx, (lo, hi, a_out_q) in enumerate(src_b):
                    recv = recv_pool.tile([WC, ncores, RB], DTM,
                                          name=f"recv{idx}")
                    recvs.append(recv)
                    nc.sync.dma_start(
                        out=recv, in_=a_out_q.rearrange("j p q -> p j q")
                    )
                # half-valid collective outputs carry zeros in their invalid
                # shards, so summing the matmul contributions of every recv
                # buffer is exact - and the first buffer's matmuls can run
                # while the later collective is still in flight.
                nsteps = len(recvs) * ncores
                for m in range(MT):
                    for ncol in range(NCOL):
                        yp = st_pool.tile([128, 512], F32, tag="st")
                        step = 0
                        for recv in recvs:
                            for j in range(ncores):
                                nc.tensor.matmul(
                                    yp,
                                    lhsT=recv[:, j, m * 128:(m + 1) * 128],
                                    rhs=wp_sb[:, j,
                                              ncol * 512:(ncol + 1) * 512],
                                    start=(step == 0),
                                    stop=(step == nsteps - 1),
                                )
                                step += 1
                        y_sb = y_pool.tile([128, 512], F32)
                        nc.vector.tensor_add(
                            y_sb, yp, b_sb[:, ncol * 512:(ncol + 1) * 512]
                        )
                        nc.sync.dma_start(
                            out=out[
                                bb,
                                m * 128:(m + 1) * 128,
                                ncol * 512:(ncol + 1) * 512,
                            ],
                            in_=y_sb,
                        )

            for b in range(b_sz):
                # ---- QKV projection: Q^T, K^T, V^T strips [WC, n_sz] ----
                QT = qt_pool.tile([WC, n_sz], DTM)
                KTs = kt_pool.tile([WC, n_sz], DTM)
                VT = vt_pool.tile([WC, n_sz], DTT)
                for rc in range(NRC):
                    x_sb = xpool.tile([128, CCH, RCW], DTM)
                    nc.sync.dma_start(
                        out=x_sb,
                        in_=xT[
                            :, b * n_sz + rc * RCW: b * n_sz + (rc + 1) * RCW
                        ].rearrange("(cc p) q -> p cc q", p=128),
                    )
                    # three concurrently-live accumulation chains in three
                    # different PSUM banks, interleaved cc-wise so
                    # consecutive matmuls hit different banks (fill/drain
                    # overlap instead of per-matmul latency exposure). V's
                    # chain borrows a po slot (idle during this phase).
                    strips = ((QT, 0, st_pool, "st"), (KTs, WC, st_pool, "st"),
                              (VT, 2 * WC, po_pool, "po"))
                    pss = [pool.tile([128, RCW], F32, tag=tag,
                                     name=f"qkvps{si}")
                           for si, (_, _, pool, tag) in enumerate(strips)]
                    for cc in range(CCH):
                        for si, (strip, base, _, _) in enumerate(strips):
                            nc.tensor.matmul(
                                pss[si],
                                lhsT=w_sb[:, cc, base:base + WC],
                                rhs=x_sb[:, cc, :],
                                start=(cc == 0),
                                stop=(cc == CCH - 1),
                            )
                    for si, (strip, base, _, _) in enumerate(strips):
                        nc.vector.tensor_copy(
                            strip[:, rc * RCW:(rc + 1) * RCW], pss[si]
                        )

                # ---- V^T -> V natural (PE transpose), ones column appended ----
                Vaug = vaug_pool.tile([128, HPC * NKT, HD + 1], DTM)
                nc.vector.memset(Vaug[:, :, HD], 1.0)
                for kt in range(NKT):
                    # one full 128x128 transpose covers both heads' d-rows;
                    # columns 0:64 / 64:128 of the result are V_h0 / V_h1
                    pst = st_pool.tile([128, 128], DTT, tag="st")
                    nc.tensor.transpose(
                        pst, VT[:, kt * 128:(kt + 1) * 128], identity
                    )
                    nc.vector.tensor_copy(
                        Vaug[:, kt::NKT, 0:HD], pst.rearrange(
                            "p (h d) -> p h d", h=HPC
                        )
                    )

                # ---- attention per head ----
                last_b = b == b_sz - 1
                if not last_b:
                    a_in = dram_in.tile([ncores, WC, RB], DTM, tag="ain")
                if last_b:
                    a2a_outs[b] = []
                oThs = [ot_pool.tile([HD, n_sz], DTM, name=f"oTh{h}")
                        for h in range(HPC)]
                # Schraudolph fast-exp constants for the DVE path: bf16 has
                # an 8-bit exponent / 7-bit mantissa, so
                # bitcast_bf16(int16(s/ln2 * 2^7 + (127 - c) * 2^7)) ~ exp(s)
                # (~2% sawtooth error, softmax-normalization cancels most of
                # it). The DVE output convert TRUNCATES, hence the +0.5.
                SCH_A = (2.0 ** 7) / float(np.log(2.0)) * SCALE
                SCH_B = (2.0 ** 7) * (127.0 - 0.0579) + 0.5
                for qc in range(NQC):
                    pos = [po_pool.tile([HD + 1, QCW], F32, tag="po",
                                        name=f"po{h}")
                           for h in range(HPC)]
                    for kt in range(NKT):
                        sts = [st_pool.tile([128, QCW], F32, tag="st",
                                            name=f"st{h}")
                               for h in range(HPC)]
                        for hf in range(QCW // 512):
                            q0 = qc * QCW + hf * 512
                            # adjacent same-shape matmuls on disjoint
                            # row-groups (head 0: partitions 0:64, head 1:
                            # 64:128) execute concurrently in the PE array
                            for h in range(HPC):
                                nc.tensor.matmul(
                                    sts[h][:, hf * 512:(hf + 1) * 512],
                                    lhsT=KTs[h * HD:(h + 1) * HD,
                                             kt * 128:(kt + 1) * 128],
                                    rhs=QT[h * HD:(h + 1) * HD, q0:q0 + 512],
                                    start=True,
                                    stop=True,
                                )
                        # exp split across engines (different PSUM tiles):
                        # ScalarE exps head 0 while VectorE fast-exps head 1
                        es0 = es_pool.tile([128, QCW], DTM, name="es0")
                        nc.scalar.activation(
                            es0,
                            sts[0],
                            mybir.ActivationFunctionType.Exp,
                            scale=SCALE,
                        )
                        es1i = es_pool.tile([128, QCW], mybir.dt.int16,
                                            name="es1")
                        nc.vector.tensor_scalar(
                            es1i,
                            sts[1],
                            SCH_A,
                            SCH_B,
                            mybir.AluOpType.mult,
                            mybir.AluOpType.add,
                        )
                        ess = [es0, es1i.bitcast(DTM)]
                        for h in range(HPC):
                            for hf in range(QCW // 512):
                                nc.tensor.matmul(
                                    pos[h][:, hf * 512:(hf + 1) * 512],
                                    lhsT=Vaug[:, h * NKT + kt, :],
                                    rhs=ess[h][:, hf * 512:(hf + 1) * 512],
                                    start=(kt == 0),
                                    stop=(kt == NKT - 1),
                                )
                    # epilogue (PE-free): copy the numerator out of PSUM on
                    # ScalarE right away (together with Ln this releases the
                    # po banks early, so the next q-chunk's PV matmuls don't
                    # stall on the epilogue tail); 1/sums = exp(-ln(sums)) on
                    # ACT; the reciprocal row is broadcast across partitions
                    # by a stride-0 DMA on the idle GpSimd queue; normalize
                    # on DVE.
                    for h in range(HPC):
                        po = pos[h]
                        num_sb = bc_pool.tile([HD, QCW], F32, name="num")
                        nc.scalar.copy(num_sb, po[0:HD, :])
                        lnr = eps_pool.tile([HD + 1, QCW], F32, tag="eps")
                        nc.scalar.activation(
                            lnr[HD:HD + 1, :],
                            po[HD:HD + 1, :],
                            mybir.ActivationFunctionType.Ln,
                        )
                        rsr = eps_pool.tile([HD + 1, QCW], F32, tag="epsb")
                        nc.scalar.activation(
                            rsr[HD:HD + 1, :],
                            lnr[HD:HD + 1, :],
                            mybir.ActivationFunctionType.Exp,
                            scale=-1.0,
                        )
                        # broadcast the reciprocal row across partitions via
                        # a DRAM bounce (SBUF src DMAs can't have stride-0
                        # partition dims; DRAM srcs can, like the bias load)
                        rs_dram = eps_dram.tile([1, QCW], F32, name="rsd")
                        nc.gpsimd.dma_start(
                            out=rs_dram, in_=rsr[HD:HD + 1, :]
                        )
                        bc = bc_pool.tile([HD, QCW], F32)
                        nc.gpsimd.dma_start(
                            out=bc,
                            in_=bass.AP(
                                tensor=rs_dram.tensor,
                                offset=rs_dram.offset,
                                ap=[[0, HD]] + list(rs_dram.ap)[1:],
                            ),
                        )
                        nc.vector.tensor_mul(
                            oThs[h][:, qc * QCW:(qc + 1) * QCW],
                            num_sb,
                            bc,
                        )
                    if last_b:
                        # the q-columns finished in this chunk are exactly
                        # the shards destined for cores
                        # [qc*jpq, (qc+1)*jpq): run a half-valid AllToAll
                        # now so only the final chunk's collective sits in
                        # the kernel tail. Other shard slots carry copies
                        # (transported but ignored by the projection).
                        jpq = QCW // RB
                        a_in_q = dram_in.tile([ncores, WC, RB], DTM,
                                              tag="ainh", name=f"ainq{qc}")
                        for h in range(HPC):
                            srcv = oThs[h][:, qc * QCW:(qc + 1) * QCW]
                            nc.sync.dma_start(
                                out=a_in_q[
                                    qc * jpq:(qc + 1) * jpq,
                                    h * HD:(h + 1) * HD, :
                                ].rearrange("j d q -> d j q"),
                                in_=srcv.rearrange("d (j q) -> d j q", j=jpq),
                            )
                        for j0 in range(0, ncores, jpq):
                            if j0 == qc * jpq:
                                continue
                            nc.sync.dma_start(
                                out=a_in_q[j0:j0 + jpq, :, :].rearrange(
                                    "j p q -> p j q"
                                ),
                                in_=bass.AP(
                                    tensor=zero_sb.tensor,
                                    offset=zero_sb.offset,
                                    ap=[list(zero_sb.ap[0]), [0, jpq]]
                                    + [list(zero_sb.ap[1])],
                                ),
                            )
                        a_out_q = dram_out.tile([ncores, WC, RB], DTM,
                                                tag="aouth",
                                                name=f"aoutq{qc}")
                        a2a_outs[b].append(
                            (qc * jpq, qc * jpq + jpq, a_out_q)
                        )
                        nc.gpsimd.collective_compute(
                            "AllToAll",
                            mybir.AluOpType.bypass,
                            replica_groups=[list(range(ncores))],
                            ins=[a_in_q.opt()],
                            outs=[a_out_q.opt()],
                        )

                for h in range(HPC):
                    oTh = oThs[h]
                    # ship this head's slice into the A2A input buffer
                    if not last_b:
                        nc.sync.dma_start(
                            out=a_in[:, h * HD:(h + 1) * HD, :].rearrange(
                                "j d q -> d j q"
                            ),
                            in_=oTh.rearrange("d (j q) -> d j q", j=ncores),
                        )

                if not last_b:
                    a_out = dram_out.tile([ncores, WC, RB], DTM, tag="aout")
                    a2a_outs[b] = a_out
                    nc.gpsimd.collective_compute(
                        "AllToAll",
                        mybir.AluOpType.bypass,
                        replica_groups=[list(range(ncores))],
                        ins=[a_in.opt()],
                        outs=[a_out.opt()],
                    )

                if b >= 2:
                    proj_phase(b - 2)
            for bb in range(max(0, b_sz - 2), b_sz):
                proj_phase(bb)

    nc.compile()
    return nc


def shard_inputs(x, w_qkv, w_proj, b_proj, b_sz=B, n_sz=N, c_sz=C, h_sz=H,
                 ncores=NCORES, dtype_mode=None):
    """Build per-core input maps from the full inputs."""
    import ml_dtypes

    dtype_mode = dtype_mode or DTYPE_MODE
    mm_np = ml_dtypes.bfloat16 if dtype_mode == "bf16" else np.float32
    HPC = h_sz // ncores
    HD = c_sz // h_sz
    x = np.asarray(x, dtype=np.float32)
    w_qkv = np.asarray(w_qkv, dtype=np.float32).astype(mm_np)
    w_proj = np.ascontiguousarray(np.asarray(w_proj, dtype=np.float32)
                                  .astype(mm_np))
    b_proj = np.ascontiguousarray(np.asarray(b_proj, dtype=np.float32))

    xT = np.ascontiguousarray(x.reshape(b_sz * n_sz, c_sz).T.astype(mm_np))
    w4 = w_qkv.reshape(c_sz, 3, h_sz, HD)
    in_maps = []
    for c in range(ncores):
        wc = np.ascontiguousarray(
            w4[:, :, c * HPC:(c + 1) * HPC, :].reshape(c_sz, 3 * HPC * HD)
        )
        in_maps.append(
            {"xT": xT, "wqkv": wc, "wproj": w_proj, "bproj": b_proj}
        )
    return in_maps


def assemble_output(results, b_sz=B, n_sz=N, c_sz=C, ncores=NCORES):
    RB = n_sz // ncores
    full = np.empty((b_sz, n_sz, c_sz), dtype=np.float32)
    for r in range(ncores):
        full[:, r * RB:(r + 1) * RB, :] = results[r]["out"]
    return full


def run(x, w_qkv, w_proj, b_proj, trace=False, **run_kwargs):
    nc = build_nc()
    in_maps = shard_inputs(x, w_qkv, w_proj, b_proj)
    last_err = None
    for attempt in range(3):
        try:
            res = run_bass_kernel_spmd(
                nc, in_maps, core_ids=list(range(NCORES)), trace=trace,
                **run_kwargs
            )
            return assemble_output(res.results), res
        except Exception as e:  # transient device wedges happen; retry
            last_err = e
            import time
            time.sleep(10)
    raise last_err


def kernel(x, w_qkv, w_proj, b_proj):
    out, _ = run(x, w_qkv, w_proj, b_proj)
    return out



# revision 23
# speedup vs baseline: 1.3189x; 1.3189x over previous
"""Multi-head attention (B=4, N=2048, C=1024, H=16) on 8 TRN2 NeuronCores.

Sharding: tensor-parallel over heads. Each core owns H/8 = 2 heads:
  - qkv projection: w_qkv column-sharded by head group, x replicated
    (passed pre-transposed as xT so the contraction dim lands on
    partitions). Q^T/K^T/V^T strips come out of 512-wide matmuls; V is
    re-transposed to natural layout on the PE (head-paired transposes run
    concurrently on disjoint row-groups), with a ones column appended.
  - attention: fully local per (batch, head). Scores are computed
    transposed (S^T[k, q]) so softmax sums reduce over the partition axis
    via the ones-column of V (PV output row 64 = softmax sums). Both
    heads' scores for one k-tile land in a single [128, 2048] bf16 PSUM
    tile via two 1024-wide matmuls emitted adjacently (disjoint PE row
    halves -> concurrent). Softmax exp is split across engines: ScalarE
    exps head 0's bank while VectorE computes head 1's exp with a
    Schraudolph bit-trick (round(a*s+b) as int16, bitcast to bf16) on the
    other PSUM bank. Softmax reciprocal is exp(-ln(sums)) on ScalarE
    (both functions pinned to one activation table set); the row is
    broadcast across partitions by a tiny ones-outer-product matmul and
    applied on VectorE.
  - projection: a per-batch AllToAll redistributes the attention output
    from column shards (head groups) to row shards (sequence slices);
    each core then computes its row slice of out @ w_proj + b_proj with
    the full w_proj replicated. The last batch instead runs one half-valid
    AllToAll per 1024-column chunk (invalid shards zero-filled, outputs
    summed via an accumulating DMA) so the kernel tail only waits on the
    final chunk's collective.

Everything is emitted under one TileContext; batches are software-
pipelined (qkv/attn of batch b overlap the collective of batch b-1 and
the projection of batch b-2).
"""


import numpy as np

import concourse.bass as bass
import concourse.mybir as mybir
import concourse.tile as tile
from concourse import bacc
from concourse.bass_utils import run_bass_kernel_spmd
from concourse.masks import make_identity

F32 = mybir.dt.float32
F32R = mybir.dt.float32r
BF16 = mybir.dt.bfloat16

DTYPE_MODE = "bf16"   # "bf16" | "f32r"

# Problem shape (hardcoded for the harness; build_nc is parameterized for
# small-shape simulation in test.py).
B, N, C, H = 4, 2048, 1024, 16
NCORES = 8


def _raw_matmul(nc, out, lhsT, rhs, start, stop):
    """nc.tensor.matmul minus the fp32-only output assert, so S^T scores
    can drain from PSUM directly as bf16 (1024-wide, one bank)."""
    eng = nc.tensor
    ifmap_ap = eng.lower_ap(rhs.opt({0}), opt=False)
    weights_ap = eng.lower_ap(lhsT.opt({0}), opt=False,
                              for_matmul_weights=True)
    out_ap = eng.lower_ap(out)

    def round_up_size(size):
        for v in (32, 64, 128):
            if v >= size:
                return v
        raise AssertionError(size)

    assert lhsT.base_partition() == rhs.base_partition()
    tile_position = (lhsT.base_partition(), out.base_partition())
    tile_size = (round_up_size(rhs.partition_size()),
                 round_up_size(out.partition_size()))
    return eng.add_instruction(mybir.InstMatmult(
        name=nc.get_next_instruction_name(),
        replication_resolution=0,
        replication_shift_amnt=0,
        replication_num_rows=0,
        start_tensor_calc=start,
        stop_tensor_calc=stop,
        ins=[ifmap_ap, weights_ap],
        outs=[out_ap],
        perf_mode=None,
        is_transpose=False,
        ifmap_quant_offset=None,
        weights_quant_offset=None,
        bass_skip_group_check=False,
        tile_position=tile_position,
        tile_size=tile_size,
    ))


def build_nc(b_sz=B, n_sz=N, c_sz=C, h_sz=H, ncores=NCORES, dtype_mode=None):
    HD = c_sz // h_sz                # head dim (64)
    HPC = h_sz // ncores             # heads per core (2)
    WC = HPC * HD                    # this core's out-column width (128)
    RB = n_sz // ncores              # rows per (core, batch) after A2A (256)
    SCALE = float(HD) ** -0.5

    RCW = 512                        # qkv row-chunk width
    NRC = n_sz // RCW
    QCW = 512                        # attention q-chunk width (1 PSUM bank)
    NQC = n_sz // QCW
    CHW = min(1024, n_sz)            # last-batch A2A chunk width (columns)
    NKT = n_sz // 128                # k-tiles per sequence
    CCH = c_sz // 128                # contraction chunks over C
    MT = RB // 128                   # row tiles per (core, batch) in proj
    NCOL = c_sz // 512               # 512-wide output column chunks in proj

    assert n_sz % QCW == 0 and n_sz % RCW == 0 and RB % 128 == 0
    assert c_sz % 512 == 0 and WC == 128 and HD == 64
    assert CHW % QCW == 0 and CHW % RB == 0

    dtype_mode = dtype_mode or DTYPE_MODE
    DTM = {"bf16": BF16, "f32r": F32R}[dtype_mode]
    DTT = BF16 if dtype_mode == "bf16" else F32   # transpose-path dtype

    # Pin every activation to the one table set containing both Exp and
    # Ln, so the Ln/Exp softmax-reciprocal never thrashes ACT table loads.
    from concourse import hw_specs
    tables = hw_specs.get_activation_tables("gen3")
    for name, fns in tables.items():
        if name != "natural_log_exp_and_others":
            fns.discard(mybir.ActivationFunctionType.Exp)
            fns.discard(mybir.ActivationFunctionType.Ln)

    nc = bacc.Bacc(
        "TRN2", target_bir_lowering=False, debug=False, num_devices=ncores
    )

    xT = nc.dram_tensor("xT", [c_sz, b_sz * n_sz], DTM, kind="ExternalInput").ap()
    wqkv = nc.dram_tensor("wqkv", [c_sz, 3 * WC], DTM, kind="ExternalInput").ap()
    wproj = nc.dram_tensor("wproj", [c_sz, c_sz], DTM, kind="ExternalInput").ap()
    bproj = nc.dram_tensor("bproj", [c_sz], F32, kind="ExternalInput").ap()
    out = nc.dram_tensor("out", [b_sz, RB, c_sz], F32, kind="ExternalOutput").ap()

    with tile.TileContext(nc) as tc:
        from contextlib import ExitStack

        with ExitStack() as ctx:
            singles = ctx.enter_context(tc.tile_pool(name="singles", bufs=1))
            xpool = ctx.enter_context(tc.tile_pool(name="xpool", bufs=2))
            qt_pool = ctx.enter_context(tc.tile_pool(name="qt", bufs=2))
            kt_pool = ctx.enter_context(tc.tile_pool(name="kt", bufs=2))
            vt_pool = ctx.enter_context(tc.tile_pool(name="vt", bufs=1))
            vaug_pool = ctx.enter_context(tc.tile_pool(name="vaug", bufs=2))
            es_pool = ctx.enter_context(tc.tile_pool(name="es", bufs=2))
            ot_pool = ctx.enter_context(tc.tile_pool(name="ot", bufs=2))
            eps_pool = ctx.enter_context(tc.tile_pool(name="eps", bufs=2))
            bc_pool = ctx.enter_context(tc.tile_pool(name="bc", bufs=2))
            recv_pool = ctx.enter_context(tc.tile_pool(name="recv", bufs=2))
            y_pool = ctx.enter_context(tc.tile_pool(name="y", bufs=2))
            st_pool = ctx.enter_context(
                tc.tile_pool(name="stp", bufs=4, space="PSUM")
            )
            po_pool = ctx.enter_context(
                tc.tile_pool(name="pop", bufs=2, space="PSUM")
            )
            ix_pool = ctx.enter_context(
                tc.tile_pool(name="ixp", bufs=2, space="PSUM")
            )
            dram_in = ctx.enter_context(
                tc.tile_pool(name="a2a_in", bufs=3, space="DRAM")
            )
            eps_dram = ctx.enter_context(
                tc.tile_pool(name="eps_dram", bufs=4, space="DRAM")
            )
            dram_out = ctx.enter_context(
                tc.tile_pool(name="a2a_out", bufs=6, space="DRAM")
            )

            # ---- constants / weights ----
            w_sb = singles.tile([128, CCH, 3 * WC], DTM)
            nc.sync.dma_start(
                out=w_sb, in_=wqkv.rearrange("(cc p) m -> p cc m", p=128)
            )
            identity = singles.tile([128, 128], DTT)
            make_identity(nc, identity)
            zero_sb = singles.tile([128, n_sz // ncores], DTM)
            nc.vector.memset(zero_sb, 0.0)

            a2a_outs = [None] * b_sz
            proj_consts = {}

            def ensure_proj_consts():
                # the projection weights are first needed two batches in -
                # defer their (2 MiB) load so batch 0's x chunks lead the
                # sync DMA queue at startup
                if proj_consts:
                    return
                wp_sb = singles.tile([128, CCH, c_sz], DTM)
                nc.sync.dma_start(
                    out=wp_sb, in_=wproj.rearrange("(cc p) m -> p cc m", p=128)
                )
                b_sb = singles.tile([128, c_sz], F32)
                nc.gpsimd.dma_start(
                    out=b_sb,
                    in_=bass.AP(
                        tensor=bproj.tensor,
                        offset=bproj.offset,
                        ap=[[0, 128]] + list(bproj.ap),
                    ),
                )
                proj_consts["wp"] = wp_sb
                proj_consts["b"] = b_sb

            def make_proj_units(bb):
                """Projection of batch bb as emit-closures (one per 128x512
                output block) for interleaving into another batch's
                attention loop. Half-valid collective outputs carry zeros in
                their invalid shards, so summing the matmul contributions of
                every recv buffer is exact."""
                state = {}

                def setup():
                    ensure_proj_consts()
                    src_b = a2a_outs[bb]
                    if not isinstance(src_b, list):
                        src_b = [(None, None, src_b)]
                    recvs = []
                    for idx, (lo, hi, a_out_q) in enumerate(src_b):
                        recv = recv_pool.tile([WC, ncores, RB], DTM,
                                              name=f"recv{idx}")
                        recvs.append(recv)
                        nc.sync.dma_start(
                            out=recv, in_=a_out_q.rearrange("j p q -> p j q")
                        )
                    state["recvs"] = recvs

                def block(m, ncol):
                    def emit():
                        if "recvs" not in state:
                            setup()
                        recvs = state["recvs"]
                        wp_sb = proj_consts["wp"]
                        b_sb = proj_consts["b"]
                        nsteps = len(recvs) * ncores
                        yp = st_pool.tile([128, 512], F32, tag="st",
                                          name="yp")
                        step = 0
                        for recv in recvs:
                            for j in range(ncores):
                                nc.tensor.matmul(
                                    yp,
                                    lhsT=recv[:, j, m * 128:(m + 1) * 128],
                                    rhs=wp_sb[:, j,
                                              ncol * 512:(ncol + 1) * 512],
                                    start=(step == 0),
                                    stop=(step == nsteps - 1),
                                )
                                step += 1
                        y_sb = y_pool.tile([128, 512], F32)
                        nc.vector.tensor_add(
                            y_sb, yp, b_sb[:, ncol * 512:(ncol + 1) * 512]
                        )
                        nc.sync.dma_start(
                            out=out[
                                bb,
                                m * 128:(m + 1) * 128,
                                ncol * 512:(ncol + 1) * 512,
                            ],
                            in_=y_sb,
                        )
                    return emit

                return [block(m, ncol)
                        for m in range(MT) for ncol in range(NCOL)]

            def proj_phase(bb):
                for u in make_proj_units(bb):
                    u()

            def make_qkv_units(b):
                """Allocate batch b's strip tiles and return them plus
                emit-closures: one per strip accumulation chain (8 matmuls
                in an ix_pool bank + a drain copy) and one per V-transpose.
                Consecutive closures land in alternating ix_pool banks so
                their fills/drains overlap."""
                QT = qt_pool.tile([WC, n_sz], DTM, name=f"QT{b}")
                KTs = kt_pool.tile([WC, n_sz], DTM, name=f"KT{b}")
                VT = vt_pool.tile([WC, n_sz], DTT, name=f"VT{b}")
                Vaug = vaug_pool.tile([128, HPC * NKT, HD + 1], DTM,
                                      name=f"Vaug{b}")
                nc.vector.memset(Vaug[:, :, HD], 1.0)
                xs = {}

                def chain(rc, base, strip, load_x):
                    def emit():
                        if load_x:
                            x_sb = xpool.tile([128, CCH, RCW], DTM,
                                              name="xsb")
                            nc.sync.dma_start(
                                out=x_sb,
                                in_=xT[
                                    :,
                                    b * n_sz + rc * RCW:
                                    b * n_sz + (rc + 1) * RCW,
                                ].rearrange("(cc p) q -> p cc q", p=128),
                            )
                            xs[rc] = x_sb
                        x_sb = xs[rc]
                        ps = ix_pool.tile([128, RCW], F32, tag="ix",
                                          name="qkvps")
                        for cc in range(CCH):
                            nc.tensor.matmul(
                                ps,
                                lhsT=w_sb[:, cc, base:base + WC],
                                rhs=x_sb[:, cc, :],
                                start=(cc == 0),
                                stop=(cc == CCH - 1),
                            )
                        nc.vector.tensor_copy(
                            strip[:, rc * RCW:(rc + 1) * RCW], ps
                        )
                    return emit

                def vtrans(kt):
                    # one full 128x128 transpose covers both heads' d-rows;
                    # columns 0:64 / 64:128 of the result are V_h0 / V_h1
                    def emit():
                        pst = ix_pool.tile([128, 128], DTT, tag="ix",
                                           name="pst")
                        nc.tensor.transpose(
                            pst, VT[:, kt * 128:(kt + 1) * 128], identity
                        )
                        nc.vector.tensor_copy(
                            Vaug[:, kt::NKT, 0:HD],
                            pst.rearrange("p (h d) -> p h d", h=HPC),
                        )
                    return emit

                units = []
                kt_per_rc = RCW // 128
                for rc in range(NRC):
                    for si, base in enumerate((0, WC, 2 * WC)):
                        strip = (QT, KTs, VT)[si]
                        units.append(chain(rc, base, strip, si == 0))
                    for kt in range(rc * kt_per_rc, (rc + 1) * kt_per_rc):
                        units.append(vtrans(kt))
                return (QT, KTs, Vaug), units

            # batch 0's QKV runs as a prologue; later batches' QKV and the
            # projection of batch b-2 are emitted interleaved into batch b's
            # attention loop, so the in-order PE queue always has
            # independent full-width matmuls between the exp-dependent
            # attention ones.
            cur_strips, units0 = make_qkv_units(0)
            for u in units0:
                u()
            for b in range(b_sz):
                QT, KTs, Vaug = cur_strips
                pending = []
                nxt_strips = None
                if b + 1 < b_sz:
                    nxt_strips, nxt_units = make_qkv_units(b + 1)
                    pending += nxt_units
                if b >= 2:
                    pending += make_proj_units(b - 2)
                nslots = NQC * NKT
                stride = max(1, nslots // max(1, len(pending)))
                slot = 0

                # ---- attention per head ----
                last_b = b == b_sz - 1
                if not last_b:
                    a_in = dram_in.tile([ncores, WC, RB], DTM, tag="ain")
                if last_b:
                    a2a_outs[b] = []
                oThs = [ot_pool.tile([HD, n_sz], DTM, name=f"oTh{h}")
                        for h in range(HPC)]
                # Schraudolph fast-exp constants for the DVE path: bf16 has
                # an 8-bit exponent / 7-bit mantissa, so
                # bitcast_bf16(int16(s/ln2 * 2^7 + (127 - c) * 2^7)) ~ exp(s)
                # (~2% sawtooth error, softmax-normalization cancels most of
                # it). The DVE output convert TRUNCATES, hence the +0.5.
                SCH_A = (2.0 ** 7) / float(np.log(2.0)) * SCALE
                SCH_B = (2.0 ** 7) * (127.0 - 0.0579) + 0.5
                for qc in range(NQC):
                    pos = [po_pool.tile([HD + 1, QCW], F32, tag="po",
                                        name=f"po{h}")
                           for h in range(HPC)]
                    for kt in range(NKT):
                        sts = [st_pool.tile([128, QCW], F32, tag="st",
                                            name=f"st{h}")
                               for h in range(HPC)]
                        for hf in range(QCW // 512):
                            q0 = qc * QCW + hf * 512
                            # adjacent same-shape matmuls on disjoint
                            # row-groups (head 0: partitions 0:64, head 1:
                            # 64:128) execute concurrently in the PE array
                            for h in range(HPC):
                                nc.tensor.matmul(
                                    sts[h][:, hf * 512:(hf + 1) * 512],
                                    lhsT=KTs[h * HD:(h + 1) * HD,
                                             kt * 128:(kt + 1) * 128],
                                    rhs=QT[h * HD:(h + 1) * HD, q0:q0 + 512],
                                    start=True,
                                    stop=True,
                                )
                        # exp split across engines (different PSUM tiles):
                        # ScalarE exps head 0 while VectorE fast-exps head 1
                        es0 = es_pool.tile([128, QCW], DTM, name="es0")
                        nc.scalar.activation(
                            es0,
                            sts[0],
                            mybir.ActivationFunctionType.Exp,
                            scale=SCALE,
                        )
                        es1i = es_pool.tile([128, QCW], mybir.dt.int16,
                                            name="es1")
                        nc.vector.tensor_scalar(
                            es1i,
                            sts[1],
                            SCH_A,
                            SCH_B,
                            mybir.AluOpType.mult,
                            mybir.AluOpType.add,
                        )
                        ess = [es0, es1i.bitcast(DTM)]
                        for h in range(HPC):
                            for hf in range(QCW // 512):
                                nc.tensor.matmul(
                                    pos[h][:, hf * 512:(hf + 1) * 512],
                                    lhsT=Vaug[:, h * NKT + kt, :],
                                    rhs=ess[h][:, hf * 512:(hf + 1) * 512],
                                    start=(kt == 0),
                                    stop=(kt == NKT - 1),
                                )
                        if pending and slot % stride == stride - 1:
                            pending.pop(0)()
                        slot += 1
                    # epilogue (PE-free): copy the numerator out of PSUM on
                    # ScalarE right away (together with Ln this releases the
                    # po banks early, so the next q-chunk's PV matmuls don't
                    # stall on the epilogue tail); 1/sums = exp(-ln(sums)) on
                    # ACT; the reciprocal row is broadcast across partitions
                    # by a stride-0 DMA on the idle GpSimd queue; normalize
                    # on DVE.
                    for h in range(HPC):
                        po = pos[h]
                        num_sb = bc_pool.tile([HD, QCW], F32, name="num")
                        nc.scalar.copy(num_sb, po[0:HD, :])
                        lnr = eps_pool.tile([HD + 1, QCW], F32, tag="eps",
                                             bufs=1)
                        nc.scalar.activation(
                            lnr[HD:HD + 1, :],
                            po[HD:HD + 1, :],
                            mybir.ActivationFunctionType.Ln,
                        )
                        rsr = eps_pool.tile([HD + 1, QCW], F32, tag="epsb",
                                             bufs=1)
                        nc.scalar.activation(
                            rsr[HD:HD + 1, :],
                            lnr[HD:HD + 1, :],
                            mybir.ActivationFunctionType.Exp,
                            scale=-1.0,
                        )
                        # broadcast the reciprocal row across partitions via
                        # a DRAM bounce (SBUF src DMAs can't have stride-0
                        # partition dims; DRAM srcs can, like the bias load)
                        rs_dram = eps_dram.tile([1, QCW], F32, name="rsd")
                        nc.gpsimd.dma_start(
                            out=rs_dram, in_=rsr[HD:HD + 1, :]
                        )
                        bc = bc_pool.tile([HD, QCW], F32)
                        nc.gpsimd.dma_start(
                            out=bc,
                            in_=bass.AP(
                                tensor=rs_dram.tensor,
                                offset=rs_dram.offset,
                                ap=[[0, HD]] + list(rs_dram.ap)[1:],
                            ),
                        )
                        nc.vector.tensor_mul(
                            oThs[h][:, qc * QCW:(qc + 1) * QCW],
                            num_sb,
                            bc,
                        )
                    if last_b and ((qc + 1) * QCW) % CHW == 0:
                        # the q-columns finished in this CHW-wide chunk are
                        # exactly the shards destined for cores
                        # [ch*jpq, (ch+1)*jpq): run a half-valid AllToAll
                        # now so only the final chunk's collective sits in
                        # the kernel tail. Other shard slots carry zeros
                        # (transported but harmless to the projection sum).
                        ch = ((qc + 1) * QCW) // CHW - 1
                        jpq = CHW // RB
                        ccol0 = ch * CHW
                        a_in_q = dram_in.tile([ncores, WC, RB], DTM,
                                              tag="ainh", name=f"ainq{ch}")
                        for h in range(HPC):
                            srcv = oThs[h][:, ccol0:ccol0 + CHW]
                            nc.sync.dma_start(
                                out=a_in_q[
                                    ch * jpq:(ch + 1) * jpq,
                                    h * HD:(h + 1) * HD, :
                                ].rearrange("j d q -> d j q"),
                                in_=srcv.rearrange("d (j q) -> d j q", j=jpq),
                            )
                        for j0 in range(0, ncores, jpq):
                            if j0 == ch * jpq:
                                continue
                            nc.sync.dma_start(
                                out=a_in_q[j0:j0 + jpq, :, :].rearrange(
                                    "j p q -> p j q"
                                ),
                                in_=bass.AP(
                                    tensor=zero_sb.tensor,
                                    offset=zero_sb.offset,
                                    ap=[list(zero_sb.ap[0]), [0, jpq]]
                                    + [list(zero_sb.ap[1])],
                                ),
                            )
                        a_out_q = dram_out.tile([ncores, WC, RB], DTM,
                                                tag="aouth",
                                                name=f"aoutq{ch}")
                        a2a_outs[b].append(
                            (ch * jpq, ch * jpq + jpq, a_out_q)
                        )
                        nc.gpsimd.collective_compute(
                            "AllToAll",
                            mybir.AluOpType.bypass,
                            replica_groups=[list(range(ncores))],
                            ins=[a_in_q.opt()],
                            outs=[a_out_q.opt()],
                        )

                # any interleave units not yet emitted must land before the
                # next batch's attention references their outputs
                for u in pending:
                    u()

                for h in range(HPC):
                    oTh = oThs[h]
                    # ship this head's slice into the A2A input buffer
                    if not last_b:
                        nc.sync.dma_start(
                            out=a_in[:, h * HD:(h + 1) * HD, :].rearrange(
                                "j d q -> d j q"
                            ),
                            in_=oTh.rearrange("d (j q) -> d j q", j=ncores),
                        )

                if not last_b:
                    a_out = dram_out.tile([ncores, WC, RB], DTM, tag="aout")
                    a2a_outs[b] = a_out
                    nc.gpsimd.collective_compute(
                        "AllToAll",
                        mybir.AluOpType.bypass,
                        replica_groups=[list(range(ncores))],
                        ins=[a_in.opt()],
                        outs=[a_out.opt()],
                    )

                cur_strips = nxt_strips
            for bb in range(max(0, b_sz - 2), b_sz):
                proj_phase(bb)

    nc.compile()
    return nc


def shard_inputs(x, w_qkv, w_proj, b_proj, b_sz=B, n_sz=N, c_sz=C, h_sz=H,
                 ncores=NCORES, dtype_mode=None):
    """Build per-core input maps from the full inputs."""
    import ml_dtypes

    dtype_mode = dtype_mode or DTYPE_MODE
    mm_np = ml_dtypes.bfloat16 if dtype_mode == "bf16" else np.float32
    HPC = h_sz // ncores
    HD = c_sz // h_sz
    x = np.asarray(x, dtype=np.float32)
    w_qkv = np.asarray(w_qkv, dtype=np.float32).astype(mm_np)
    w_proj = np.ascontiguousarray(np.asarray(w_proj, dtype=np.float32)
                                  .astype(mm_np))
    b_proj = np.ascontiguousarray(np.asarray(b_proj, dtype=np.float32))

    xT = np.ascontiguousarray(x.reshape(b_sz * n_sz, c_sz).T.astype(mm_np))
    w4 = w_qkv.reshape(c_sz, 3, h_sz, HD)
    in_maps = []
    for c in range(ncores):
        wc = np.ascontiguousarray(
            w4[:, :, c * HPC:(c + 1) * HPC, :].reshape(c_sz, 3 * HPC * HD)
        )
        in_maps.append(
            {"xT": xT, "wqkv": wc, "wproj": w_proj, "bproj": b_proj}
        )
    return in_maps


def assemble_output(results, b_sz=B, n_sz=N, c_sz=C, ncores=NCORES):
    RB = n_sz // ncores
    full = np.empty((b_sz, n_sz, c_sz), dtype=np.float32)
    for r in range(ncores):
        full[:, r * RB:(r + 1) * RB, :] = results[r]["out"]
    return full


def run(x, w_qkv, w_proj, b_proj, trace=False, **run_kwargs):
    nc = build_nc()
    in_maps = shard_inputs(x, w_qkv, w_proj, b_proj)
    last_err = None
    for attempt in range(3):
        try:
            res = run_bass_kernel_spmd(
                nc, in_maps, core_ids=list(range(NCORES)), trace=trace,
                **run_kwargs
            )
            return assemble_output(res.results), res
        except Exception as e:  # transient device wedges happen; retry
            last_err = e
            import time
            time.sleep(10)
    raise last_err


def kernel(x, w_qkv, w_proj, b_proj):
    out, _ = run(x, w_qkv, w_proj, b_proj)
    return out

